# revision 53
# baseline (speedup 1.0000x reference)
"""Tensor-parallel GQA multi-head-attention kernel for 8 trn2 NeuronCores.

Problem: B=2, T=2048, D=2048, H=16 q-heads, KV=4 kv-heads, HD=128,
causal attention with interleaved RoPE, y = attn_out @ Wo.

Sharding (tensor-parallel over heads, per the hint):
  core c = b*4 + g   (b = batch index, g = kv-head / q-head-group index)
  Each core computes q-heads 4g..4g+3 and kv-head g for batch b, plus the
  partial output  y_partial = attn_heads @ Wo[rows of those heads]  (row-
  parallel Wo).  The host sums the 4 partials per batch (the unshard of the
  row-parallel all-reduce) and stacks the 2 batches.

On-chip design (per core, everything bf16 except PSUM/softmax math):
  - host pre-arranges all inputs so every load is one (or a few) fully
    contiguous-per-partition DMA; bulk loads ride the two HWDGE rings
    (sync/scalar) in exact consumption order, small tables ride the gpsimd
    SWDGE ring.  The input stream is HBM-bound (~300 GB/s) for the first
    ~50us, so ordering is what keeps the PE fed.
  - dummy matmuls (never read) pad the DMA-paced start so the HAM clock
    gate warms at ~11us and real matmuls run at 2.4 GHz, not 1.2.
  - rope tables are built on device by angle addition from 0.5 MB of host
    tables (cos/sin of invf*u, u<512, plus per-chunk A-scalars), removing
    1 MB from the critical load path.
  - projections: q^T[h] = Wq_h^T @ xT  (lhsT=Wq chunk), k^T likewise;
    v^T the same way, then turned into v-natural via PE transposes.
  - RoPE (3 DVE ops): qf = psum*cos_dup; qs_pre = psum*ssw (sin signs
    pre-swapped); SBUF half-swap DMA of qs_pre; dst = qf + qs.
  - attention per (head, 512-wide q chunk): for each 128-row k tile
    S^T = k^T_tile.T(dot) q^T chunk -> PSUM [128,512]; diagonal blocks get a
    -30000 mask add (DVE); ACT computes P = exp(scale*S^T) -> SBUF bf16;
    PV accumulates out^T[HD,512] with lhsT = v tile; an all-ones [128,128]
    lhsT matmul accumulates the softmax denominators broadcast across all
    128 partitions; normalization = reciprocal + one DVE multiply.
    Fully-masked (future) blocks are skipped -> ~40% less attention work.
  - Wo: y tile [128,512] = sum_h attnT_h chunk.T @ Wo_h chunk, copied bf16
    into a [128,2048] row tile, one DMA per row tile; the final tile ships
    as 4 pipelined 512-col DMAs on the fast rings to shorten the tail.
  - y partials are written bf16 (halves write traffic; adds ~0.2% error,
    total rel err ~5.4e-3 vs the 2e-2 gate).
"""

import math
import sys

import numpy as np

for _p in ("/opt/trn_rl_repo", "/root/.axon_site",
           "/root/.axon_site/_ro/trn_rl_repo",
           "/root/.axon_site/_ro/pypackages"):
    if _p not in sys.path:
        sys.path.append(_p)

B, T, D = 2, 2048, 2048
H, KV, HD = 16, 4, 128
ROPE_BASE = 10000.0
N_CORES = 8
HEADS_PER_CORE = 4
DQ = HEADS_PER_CORE * HD  # 512 q-dims per core
SCALE = 1.0 / math.sqrt(HD)
MASK_VAL = -30000.0

_CACHE = {}


def _build_nc(t_len=T):
    """Build the single-core SPMD Bass/Tile program (cached)."""
    import concourse.bass as bass
    import concourse.mybir as mybir
    import concourse.tile as tile
    from concourse import bacc

    f32 = mybir.dt.float32
    bf16 = mybir.dt.bfloat16
    ts = bass.ts

    NT = t_len // 128        # number of 128-row T tiles
    NK = D // 128            # contraction chunks for projections
    NCQ = t_len // 512       # number of 512-wide q chunks

    nc = bacc.Bacc("TRN2", target_bir_lowering=False, debug=False,
                   num_devices=N_CORES)

    xT_d = nc.dram_tensor("xT", [128, NCQ, NK, 512], bf16,
                          kind="ExternalInput").ap()
    wq_d = nc.dram_tensor("wq", [128, HEADS_PER_CORE, NK, HD], bf16,
                          kind="ExternalInput").ap()
    wk_d = nc.dram_tensor("wk", [128, NK, HD], bf16, kind="ExternalInput").ap()
    wv_d = nc.dram_tensor("wv", [128, NK, HD], bf16, kind="ExternalInput").ap()
    wo_d = nc.dram_tensor("wo", [128, HEADS_PER_CORE, D], bf16,
                          kind="ExternalInput").ap()
    cosb_d = nc.dram_tensor("cosb", [128, 512], bf16, kind="ExternalInput").ap()
    sinb_d = nc.dram_tensor("sinb", [128, 512], bf16, kind="ExternalInput").ap()
    rota_d = nc.dram_tensor("rota", [128, 4, NCQ], mybir.dt.float32,
                            kind="ExternalInput").ap()
    mask_d = nc.dram_tensor("mask", [128, 128], bf16, kind="ExternalInput").ap()
    id_d = nc.dram_tensor("id128", [128, 128], bf16, kind="ExternalInput").ap()
    y_d = nc.dram_tensor("y", [t_len, D], bf16, kind="ExternalOutput").ap()

    Exp = mybir.ActivationFunctionType.Exp

    with tile.TileContext(nc) as tc:
        with (
            tc.tile_pool(name="const", bufs=1) as const,
            tc.tile_pool(name="qkv", bufs=1) as qkv,
            tc.tile_pool(name="attn", bufs=3) as attn_pool,
            tc.tile_pool(name="p", bufs=8) as p_pool,
            tc.tile_pool(name="rope", bufs=2) as rope_pool,
            tc.tile_pool(name="recip", bufs=2) as recip_pool,
            tc.tile_pool(name="y", bufs=2) as y_pool,
            tc.tile_pool(name="psum", bufs=1, space="PSUM") as psum,
        ):
            # ---- input loads: few, large, contiguous-per-partition DMAs,
            # issued round-robin on all four DMA-trigger queues so transfers
            # start in parallel and the critical chunk-0 data lands first ----
            mask_sb = const.tile([128, 128], bf16, tag="mask")
            id_sb = const.tile([128, 128], bf16, tag="id")
            xT = const.tile([128, NCQ, NK, 512], bf16, tag="xT")
            wq = const.tile([128, HEADS_PER_CORE, NK, HD], bf16, tag="wq")
            wk = const.tile([128, NK, HD], bf16, tag="wk")
            wv = const.tile([128, NK, HD], bf16, tag="wv")
            wo = const.tile([128, HEADS_PER_CORE, D], bf16, tag="wo")
            cos_sb = const.tile([128, t_len], bf16, tag="cos")
            ssw_sb = const.tile([128, t_len], bf16, tag="ssw")
            cosb_sb = const.tile([128, 512], bf16, tag="cosb")
            sinb_sb = const.tile([128, 512], bf16, tag="sinb")
            rota_sb = const.tile([128, 4, NCQ], f32, tag="rota")

            # Bulk loads ride the two HWDGE rings (sync/scalar) in the order
            # compute needs them; small tables go on the gpsimd SWDGE ring.
            # chunk-0 x and wq are split into k-group sub-loads so the first
            # projection matmuls start after ~0.5 MB instead of 4 MB.
            def xkg(i):
                return (xT[:, 0, ts(i, 4), :], xT_d[:, 0, ts(i, 4), :])

            # two HWDGE rings (sync/scalar) carry the bulk in the exact order
            # the projection k-loops consume it; the slow gpsimd SWDGE ring
            # carries the small tables it has time for.  wq is loaded per
            # head so attention on head 0 starts as early as possible.
            nc.gpsimd.dma_start(cosb_sb[:], cosb_d[:])
            nc.gpsimd.dma_start(sinb_sb[:], sinb_d[:])
            nc.gpsimd.dma_start(rota_sb[:], rota_d[:])
            nc.sync.dma_start(wk[:], wk_d[:])
            nc.scalar.dma_start(*xkg(0))
            nc.sync.dma_start(*xkg(1))
            nc.scalar.dma_start(*xkg(2))
            nc.sync.dma_start(*xkg(3))
            nc.scalar.dma_start(wq[:, 0], wq_d[:, 0])
            nc.sync.dma_start(wq[:, 1], wq_d[:, 1])
            nc.scalar.dma_start(wq[:, 2], wq_d[:, 2])
            nc.sync.dma_start(wq[:, 3], wq_d[:, 3])
            nc.gpsimd.dma_start(wv[:], wv_d[:])
            nc.gpsimd.dma_start(id_sb[:], id_d[:])
            nc.gpsimd.dma_start(mask_sb[:], mask_d[:])
            nc.sync.dma_start(wo[:, 0:2, :], wo_d[:, 0:2, :])
            nc.scalar.dma_start(wo[:, 2:4, :], wo_d[:, 2:4, :])
            nc.scalar.dma_start(xT[:, 1, 0:8, :], xT_d[:, 1, 0:8, :])
            nc.sync.dma_start(xT[:, 1, 8:16, :], xT_d[:, 1, 8:16, :])
            nc.sync.dma_start(xT[:, 2], xT_d[:, 2])
            nc.scalar.dma_start(xT[:, 3], xT_d[:, 3])

            ones_sb = const.tile([128, 128], bf16, tag="ones")
            nc.vector.memset(ones_sb[:], 1.0)

            # rope tables via angle addition (no big table loads, no Sin):
            # cos(A+B) = cosA*cosB - sinA*sinB with A = invf*512c (host
            # scalars in rota) and B = invf*u (host [128,512] tables).
            Mul = mybir.AluOpType.mult
            for c in range(NCQ):
                tcs = slice(c * 512, (c + 1) * 512)
                t1 = rope_pool.tile([128, 512], f32, tag="qf", name=f"tc{c}a")
                t2 = rope_pool.tile([128, 512], f32, tag="qp", name=f"tc{c}b")
                nc.vector.tensor_scalar(t1[:], cosb_sb[:],
                                        rota_sb[:, 0, c:c+1], None, op0=Mul)
                nc.vector.tensor_scalar(t2[:], sinb_sb[:],
                                        rota_sb[:, 1, c:c+1], None, op0=Mul)
                nc.vector.tensor_sub(cos_sb[:, tcs], t1[:], t2[:])
                t3 = rope_pool.tile([128, 512], f32, tag="qf", name=f"tc{c}c")
                t4 = rope_pool.tile([128, 512], f32, tag="qp", name=f"tc{c}d")
                nc.vector.tensor_scalar(t3[:], cosb_sb[:],
                                        rota_sb[:, 3, c:c+1], None, op0=Mul)
                nc.vector.tensor_scalar(t4[:], sinb_sb[:],
                                        rota_sb[:, 2, c:c+1], None, op0=Mul)
                nc.vector.tensor_add(ssw_sb[:, tcs], t3[:], t4[:])

            # persistent activations
            qT = qkv.tile([128, HEADS_PER_CORE, t_len], bf16, tag="qT")
            kT = qkv.tile([128, t_len], bf16, tag="kT")
            v_sb = qkv.tile([128, NT, HD], bf16, tag="v")

            def rope_to(dst_ap, psum_tile, c):
                """RoPE a [128, 512] psum tile (rows = [even|odd] dims of one
                head, cols = T positions of chunk c) into bf16 dst_ap."""
                cs = slice(c * 512, (c + 1) * 512)
                qf = rope_pool.tile([128, 512], f32, tag="qf")
                qp = rope_pool.tile([128, 512], f32, tag="qp")
                qs = rope_pool.tile([128, 512], f32, tag="qs")
                nc.vector.tensor_mul(qf[:], psum_tile[:], cos_sb[:, cs])
                nc.vector.tensor_mul(qp[:], psum_tile[:], ssw_sb[:, cs])
                nc.gpsimd.dma_start(qs[0:64, :], qp[64:128, :])
                nc.gpsimd.dma_start(qs[64:128, :], qp[0:64, :])
                nc.vector.tensor_add(dst_ap, qf[:], qs[:])

            def kv_proj(c):
                """k^T and v projections for chunk c.  For chunk 0 the k/v
                matmuls interleave by k-group (the runtime PE queue is
                strictly in-order, so work must be emitted in data-arrival
                order) with dummy pads covering the DMA pacing."""
                cs = slice(c * 512, (c + 1) * 512)
                kp = psum.tile([128, 512], f32, tag="proj", bufs=2)
                vp = psum.tile([128, 512], f32, tag="proj", bufs=2)
                for k in range(NK):
                    nc.tensor.matmul(kp[:], wk[:, k, :], xT[:, c, k, :],
                                     start=(k == 0), stop=(k == NK - 1))
                for k in range(NK):
                    nc.tensor.matmul(vp[:], wv[:, k, :], xT[:, c, k, :],
                                     start=(k == 0), stop=(k == NK - 1))
                rope_to(kT[:, cs], kp, c)
                vT_sb = rope_pool.tile([128, 512], bf16, tag="vT")
                nc.any.tensor_copy(out=vT_sb[:], in_=vp[:])
                for tt in range(4):
                    trp_full = psum.tile([128, 512], f32, tag="proj", bufs=2,
                                         name="trp")
                    trp = trp_full.bitcast(bf16)[:, :128]
                    nc.tensor.transpose(trp, vT_sb[:, ts(tt, 128)], id_sb[:])
                    nc.any.tensor_copy(out=v_sb[:, 4 * c + tt, :], in_=trp)

            def q_proj(c, h):
                cs = slice(c * 512, (c + 1) * 512)
                qp = psum.tile([128, 512], f32, tag="proj", bufs=2)
                for k in range(NK):
                    nc.tensor.matmul(qp[:], wq[:, h, k, :], xT[:, c, k, :],
                                     start=(k == 0), stop=(k == NK - 1))
                rope_to(qT[:, h, cs], qp, c)

            def attn_head(c, h, attn_t):
                """Attention for q chunk c, head h -> attn_t[:, h, :].

                k tiles are processed in pairs sharing one [128,1024] psum
                so a single wide exp covers both (30% less ACT time, half
                the exp instructions).  Columns below each half's causal
                lo-trim hold stale psum; their exp outputs are never read."""
                nj = 4 * c + 4
                out_ps = psum.tile([128, 512], f32, tag="out", bufs=1)
                sums_ps = psum.tile([128, 512], f32, tag="sums", bufs=1)
                for jp in range(nj // 2):
                    sp = psum.tile([128, 1024], f32, tag="s", bufs=2,
                                   name="spair")
                    p = p_pool.tile([128, 1024], bf16, tag="p")
                    los = []
                    for half in range(2):
                        j = 2 * jp + half
                        # columns < o*128 of this [tk-tile, q-chunk] block
                        # are fully masked (tk > tq): skip them everywhere
                        o = j - 4 * c
                        lo = max(o, 0) * 128
                        los.append(lo)
                        qs0 = c * 512 + lo
                        base = half * 512
                        nc.tensor.matmul(sp[:, base + lo:base + 512],
                                         kT[:, ts(j, 128)],
                                         qT[:, h, qs0:(c + 1) * 512],
                                         start=True, stop=True)
                        if o >= 0:
                            nc.vector.tensor_add(
                                sp[:, base + lo:base + lo + 128],
                                sp[:, base + lo:base + lo + 128],
                                mask_sb[:])
                    nc.scalar.activation(p[:], sp[:], Exp,
                                         bias=0.0, scale=SCALE)
                    for half in range(2):
                        j = 2 * jp + half
                        lo = los[half]
                        base = half * 512
                        nc.tensor.matmul(out_ps[:, lo:], v_sb[:, j, :],
                                         p[:, base + lo:base + 512],
                                         start=(j == 0), stop=(j == nj - 1))
                        nc.tensor.matmul(sums_ps[:, lo:], ones_sb[:],
                                         p[:, base + lo:base + 512],
                                         start=(j == 0), stop=(j == nj - 1))
                rc = recip_pool.tile([128, 512], f32, tag="rc")
                nc.vector.reciprocal_approx_fast(out=rc[:], in_=sums_ps[:])
                nc.vector.tensor_mul(attn_t[:, h, :], out_ps[:], rc[:])

            yqs = [nc.sync, nc.scalar, nc.gpsimd]

            def wo_tile(c, tq, attn_t, split_dma=False):
                """Output projection for row tile tq of q chunk c."""
                row0 = (4 * c + tq) * 128
                # keep the final chunk's DMAs off the slow gpsimd ring so
                # the end-of-kernel queue drain doesn't stretch the tail
                nq = 2 if c == NCQ - 1 else 3
                ysb = y_pool.tile([128, D], bf16, tag="y")
                for half in range(2):
                    # two psum tiles per stationary load: consecutive
                    # matmuls share lhsT so LDWEIGHTS fully hides
                    yp0 = psum.tile([128, 512], f32, tag="proj", bufs=2,
                                    name="yp0")
                    yp1 = psum.tile([128, 512], f32, tag="proj", bufs=2,
                                    name="yp1")
                    for h in range(HEADS_PER_CORE):
                        for sub, yp in ((0, yp0), (1, yp1)):
                            nn = 2 * half + sub
                            nc.tensor.matmul(yp[:],
                                             attn_t[:, h, ts(tq, 128)],
                                             wo[:, h, ts(nn, 512)],
                                             start=(h == 0), stop=(h == 3))
                    for sub, yp in ((0, yp0), (1, yp1)):
                        nn = 2 * half + sub
                        if split_dma:
                            # tail: copies on alternating engines, ship each
                            # 512-col piece as soon as its copy lands
                            if nn % 2 == 0:
                                nc.vector.tensor_copy(
                                    out=ysb[:, ts(nn, 512)], in_=yp[:])
                            else:
                                nc.scalar.copy(ysb[:, ts(nn, 512)], yp[:])
                            yqs[nn % nq].dma_start(
                                y_d[row0:row0 + 128, ts(nn, 512)],
                                ysb[:, ts(nn, 512)])
                        else:
                            nc.vector.tensor_copy(out=ysb[:, ts(nn, 512)],
                                                  in_=yp[:])
                if not split_dma:
                    yqs[(4 * c + tq) % nq].dma_start(
                        y_d[row0:row0 + 128, :], ysb[:])

            # Emission = scheduler priority order: within a chunk, each
            # head's q-projection immediately precedes its attention so the
            # first exp fires ~35us in; the previous chunk's Wo tiles ride
            # between heads as always-ready PE filler for the ACT-paced
            # attention stretches.
            # Dummy matmuls: always-ready PE filler for the DMA-paced start.
            # They pad the arrival gaps so the HAM clock-gate warms early and
            # stays warm; results are never used (one dummy reader each).
            dmy_in = const.tile([128, 512], bf16, tag="dmy")
            nc.vector.memset(dmy_in[:], 0.0)
            dmy_rd = rope_pool.tile([128, 512], f32, tag="dmyrd")

            def dummy_mms(n, tag):
                dp = psum.tile([128, 512], f32, tag="sums", bufs=1,
                               name=f"dmy{tag}")
                for i in range(n):
                    nc.tensor.matmul(dp[:], ones_sb[:], dmy_in[:],
                                     start=(i == 0), stop=(i == n - 1))
                nc.vector.tensor_copy(out=dmy_rd[:], in_=dp[:])

            attn_ts = [None] * NCQ
            pads = {0: 8, 1: 10, 2: 12, 3: 8}
            for c in range(NCQ):
                cs = slice(c * 512, (c + 1) * 512)
                if c == 0:
                    dummy_mms(14, "warm")
                kv_proj(c)
                if c == 0:
                    dummy_mms(8, "pad0")
                attn_ts[c] = attn_pool.tile([128, HEADS_PER_CORE, 512], bf16,
                                            tag="attnT", name=f"attn_t{c}")
                for h in range(HEADS_PER_CORE):
                    q_proj(c, h)
                    if c == 0:
                        dummy_mms(pads[h], f"padq{h}")
                for h in range(HEADS_PER_CORE):
                    attn_head(c, h, attn_ts[c])
                for tq in range(4):
                    wo_tile(c, tq, attn_ts[c],
                            split_dma=(c == NCQ - 1 and tq == 3))

    nc.finalize()
    return nc


def _prep_inputs(x, Wq, Wk, Wv, Wo, t_len=T):
    """Host-side shard + layout prep -> per-core input maps."""
    import ml_dtypes
    bf16 = ml_dtypes.bfloat16

    x = np.asarray(x, np.float32)
    Wq = np.asarray(Wq, np.float32)
    Wk = np.asarray(Wk, np.float32)
    Wv = np.asarray(Wv, np.float32)
    Wo = np.asarray(Wo, np.float32)

    NK = D // 128
    NCQ = t_len // 512

    # RoPE de-interleave permutation within one head: [evens | odds]
    perm = np.concatenate([np.arange(0, HD, 2), np.arange(1, HD, 2)])

    # rope tables (match reference: freqs = t * base**(-2j/HD)), built on
    # device via angle addition from small host tables:
    #   cos_dup[p, 512c+u] = cosA[p,c]*cosB[p,u] - sinA[p,c]*sinB[p,u]
    #   ssig_sw[p, t] = sgn[p]*sin(invf[p]*t)   (sgn = [+1]*64 + [-1]*64)
    inv = 1.0 / (ROPE_BASE ** (np.arange(0, HD, 2, dtype=np.float32) / HD))
    invd = np.concatenate([inv, inv])                      # [128]
    u = np.arange(512, dtype=np.float64)
    fb = invd[:, None].astype(np.float64) * u[None, :]     # [128, 512]
    cosb = np.cos(fb).astype(bf16)
    sinb = np.sin(fb).astype(bf16)
    sgn = np.concatenate([np.ones(64), -np.ones(64)])
    rota = np.empty((128, 4, t_len // 512), np.float32)
    for c in range(t_len // 512):
        a = invd.astype(np.float64) * (512.0 * c)
        rota[:, 0, c] = np.cos(a)
        rota[:, 1, c] = np.sin(a)
        rota[:, 2, c] = sgn * np.cos(a)
        rota[:, 3, c] = sgn * np.sin(a)

    # strict-lower-triangular causal mask template for the diagonal
    # [tk-tile, tq-tile] block (tk > tq within the 128x128 block)
    r = np.arange(128)[:, None]
    col = np.arange(128)[None, :]
    mask_t = np.where(r > col, MASK_VAL, 0.0).astype(bf16)
    id128 = np.eye(128, dtype=np.float32).astype(bf16)

    def dram_kp(w):  # [D, n] -> [128, NK, n] (partition-major k-chunks)
        n = w.shape[1]
        return np.ascontiguousarray(
            w.reshape(NK, 128, n).transpose(1, 0, 2)).astype(bf16)

    in_maps = []
    for b in range(B):
        xTb = x[b, :t_len].T  # [D, T]
        xTb = np.ascontiguousarray(
            xTb.reshape(NK, 128, NCQ, 512).transpose(1, 2, 0, 3)).astype(bf16)
        for g in range(KV):
            wq_g = Wq[:, g * DQ:(g + 1) * DQ].reshape(D, HEADS_PER_CORE, HD)
            wq_g = wq_g[:, :, perm]  # [D, 4, HD]
            # per-head contiguous layout: [128, h, k, HD]
            wq_g = np.ascontiguousarray(
                wq_g.reshape(NK, 128, HEADS_PER_CORE, HD)
                .transpose(1, 2, 0, 3)).astype(bf16)
            wk_g = Wk[:, g * HD:(g + 1) * HD][:, perm]
            wv_g = Wv[:, g * HD:(g + 1) * HD]
            wo_g = Wo[g * DQ:(g + 1) * DQ, :]  # [512, D]
            wo_g = np.ascontiguousarray(
                wo_g.reshape(HEADS_PER_CORE, 128, D).transpose(1, 0, 2)
            ).astype(bf16)
            in_maps.append({
                "xT": xTb, "wq": wq_g, "wk": dram_kp(wk_g),
                "wv": dram_kp(wv_g), "wo": wo_g, "cosb": cosb,
                "sinb": sinb, "rota": rota, "mask": mask_t, "id128": id128,
            })
    return in_maps


def run(inputs, trace=False, t_len=T):
    """Run the sharded kernel; returns (y_full, BassKernelResults)."""
    from concourse.bass_utils import run_bass_kernel_spmd

    key = ("nc", t_len)
    if key not in _CACHE:
        _CACHE[key] = _build_nc(t_len)
    nc = _CACHE[key]

    in_maps = _prep_inputs(inputs["x"], inputs["Wq"], inputs["Wk"],
                           inputs["Wv"], inputs["Wo"], t_len)
    res = run_bass_kernel_spmd(nc, in_maps, list(range(N_CORES)), trace=trace)

    y = np.empty((B, t_len, D), np.float32)
    for b in range(B):
        acc = np.zeros((t_len, D), np.float32)
        for g in range(KV):
            acc += np.asarray(res.results[b * KV + g]["y"], np.float32)
        y[b] = acc
    return y, res


def kernel(**inputs) -> np.ndarray:
    y, _ = run(inputs, trace=False)
    return y


# revision 54
# speedup vs baseline: 1.2278x; 1.2278x over previous
"""Tensor-parallel GQA multi-head-attention kernel for 8 trn2 NeuronCores.

Problem: B=2, T=2048, D=2048, H=16 q-heads, KV=4 kv-heads, HD=128,
causal attention with interleaved RoPE, y = attn_out @ Wo.

Sharding (tensor-parallel over heads, per the hint):
  core c = b*4 + g   (b = batch index, g = kv-head / q-head-group index)
  Each core computes q-heads 4g..4g+3 and kv-head g for batch b, plus the
  partial output  y_partial = attn_heads @ Wo[rows of those heads]  (row-
  parallel Wo).  The host sums the 4 partials per batch (the unshard of the
  row-parallel all-reduce) and stacks the 2 batches.

On-chip design (per core, everything bf16 except PSUM/softmax math):
  - host pre-arranges all inputs so every load is one (or a few) fully
    contiguous-per-partition DMA; bulk loads ride the two HWDGE rings
    (sync/scalar) in exact consumption order, small tables ride the gpsimd
    SWDGE ring.  The input stream is HBM-bound (~300 GB/s) for the first
    ~50us, so ordering is what keeps the PE fed.
  - dummy matmuls (never read) pad the DMA-paced start so the HAM clock
    gate warms at ~11us and real matmuls run at 2.4 GHz, not 1.2.
  - rope tables are built on device by angle addition from 0.5 MB of host
    tables (cos/sin of invf*u, u<512, plus per-chunk A-scalars), removing
    1 MB from the critical load path.
  - projections: q^T[h] = Wq_h^T @ xT  (lhsT=Wq chunk), k^T likewise;
    v^T the same way, then turned into v-natural via PE transposes.
  - RoPE (3 DVE ops): qf = psum*cos_dup; qs_pre = psum*ssw (sin signs
    pre-swapped); SBUF half-swap DMA of qs_pre; dst = qf + qs.
  - attention per (head, 512-wide q chunk): for each 128-row k tile
    S^T = k^T_tile.T(dot) q^T chunk -> PSUM [128,512]; diagonal blocks get a
    -30000 mask add (DVE); ACT computes P = exp(scale*S^T) -> SBUF bf16;
    PV accumulates out^T[HD,512] with lhsT = v tile; an all-ones [128,128]
    lhsT matmul accumulates the softmax denominators broadcast across all
    128 partitions; normalization = reciprocal + one DVE multiply.
    Fully-masked (future) blocks are skipped -> ~40% less attention work.
  - Wo: y tile [128,512] = sum_h attnT_h chunk.T @ Wo_h chunk, copied bf16
    into a [128,2048] row tile, one DMA per row tile; the final tile ships
    as 4 pipelined 512-col DMAs on the fast rings to shorten the tail.
  - y partials are written bf16 (halves write traffic; adds ~0.2% error,
    total rel err ~5.4e-3 vs the 2e-2 gate).
"""

import math
import sys

import numpy as np

for _p in ("/opt/trn_rl_repo", "/root/.axon_site",
           "/root/.axon_site/_ro/trn_rl_repo",
           "/root/.axon_site/_ro/pypackages"):
    if _p not in sys.path:
        sys.path.append(_p)

B, T, D = 2, 2048, 2048
H, KV, HD = 16, 4, 128
ROPE_BASE = 10000.0
N_CORES = 8
HEADS_PER_CORE = 4
DQ = HEADS_PER_CORE * HD  # 512 q-dims per core
SCALE = 1.0 / math.sqrt(HD)
MASK_VAL = -30000.0

_CACHE = {}


def _build_nc(t_len=T):
    """Build the single-core SPMD Bass/Tile program (cached)."""
    import concourse.bass as bass
    import concourse.mybir as mybir
    import concourse.tile as tile
    from concourse import bacc

    f32 = mybir.dt.float32
    bf16 = mybir.dt.bfloat16
    ts = bass.ts

    NT = t_len // 128        # number of 128-row T tiles
    NK = D // 128            # contraction chunks for projections
    NCQ = t_len // 512       # number of 512-wide q chunks

    nc = bacc.Bacc("TRN2", target_bir_lowering=False, debug=False,
                   num_devices=N_CORES)

    xT_d = nc.dram_tensor("xT", [128, NCQ, NK, 512], bf16,
                          kind="ExternalInput").ap()
    wq_d = nc.dram_tensor("wq", [128, HEADS_PER_CORE, NK, HD], bf16,
                          kind="ExternalInput").ap()
    wk_d = nc.dram_tensor("wk", [128, NK, HD], bf16, kind="ExternalInput").ap()
    wv_d = nc.dram_tensor("wv", [128, NK, HD], bf16, kind="ExternalInput").ap()
    wo_d = nc.dram_tensor("wo", [128, HEADS_PER_CORE, D], bf16,
                          kind="ExternalInput").ap()
    cosb_d = nc.dram_tensor("cosb", [128, 512], bf16, kind="ExternalInput").ap()
    sinb_d = nc.dram_tensor("sinb", [128, 512], bf16, kind="ExternalInput").ap()
    rota_d = nc.dram_tensor("rota", [128, 4, NCQ], mybir.dt.float32,
                            kind="ExternalInput").ap()
    mask_d = nc.dram_tensor("mask", [128, 128], bf16, kind="ExternalInput").ap()
    id_d = nc.dram_tensor("id128", [128, 128], bf16, kind="ExternalInput").ap()
    y_d = nc.dram_tensor("y", [t_len, D], bf16, kind="ExternalOutput").ap()

    Exp = mybir.ActivationFunctionType.Exp

    with tile.TileContext(nc) as tc:
        with (
            tc.tile_pool(name="const", bufs=1) as const,
            tc.tile_pool(name="qkv", bufs=1) as qkv,
            tc.tile_pool(name="attn", bufs=3) as attn_pool,
            tc.tile_pool(name="p", bufs=8) as p_pool,
            tc.tile_pool(name="rope", bufs=2) as rope_pool,
            tc.tile_pool(name="recip", bufs=2) as recip_pool,
            tc.tile_pool(name="y", bufs=2) as y_pool,
            tc.tile_pool(name="psum", bufs=1, space="PSUM") as psum,
        ):
            # ---- input loads: few, large, contiguous-per-partition DMAs,
            # issued round-robin on all four DMA-trigger queues so transfers
            # start in parallel and the critical chunk-0 data lands first ----
            mask_sb = const.tile([128, 128], bf16, tag="mask")
            id_sb = const.tile([128, 128], bf16, tag="id")
            xT = const.tile([128, NCQ, NK, 512], bf16, tag="xT")
            wq = const.tile([128, HEADS_PER_CORE, NK, HD], bf16, tag="wq")
            wk = const.tile([128, NK, HD], bf16, tag="wk")
            wv = const.tile([128, NK, HD], bf16, tag="wv")
            wo = const.tile([128, HEADS_PER_CORE, D], bf16, tag="wo")
            cos_sb = const.tile([128, t_len], bf16, tag="cos")
            ssw_sb = const.tile([128, t_len], bf16, tag="ssw")
            cosb_sb = const.tile([128, 512], bf16, tag="cosb")
            sinb_sb = const.tile([128, 512], bf16, tag="sinb")
            rota_sb = const.tile([128, 4, NCQ], f32, tag="rota")

            # Bulk loads ride the two HWDGE rings (sync/scalar) in the order
            # compute needs them; small tables go on the gpsimd SWDGE ring.
            # chunk-0 x and wq are split into k-group sub-loads so the first
            # projection matmuls start after ~0.5 MB instead of 4 MB.
            def xkg(i):
                return (xT[:, 0, ts(i, 4), :], xT_d[:, 0, ts(i, 4), :])

            # two HWDGE rings (sync/scalar) carry the bulk in the exact order
            # the projection k-loops consume it; the slow gpsimd SWDGE ring
            # carries the small tables it has time for.  wq is loaded per
            # head so attention on head 0 starts as early as possible.
            nc.gpsimd.dma_start(cosb_sb[:], cosb_d[:])
            nc.gpsimd.dma_start(sinb_sb[:], sinb_d[:])
            nc.gpsimd.dma_start(rota_sb[:], rota_d[:])
            nc.sync.dma_start(wk[:], wk_d[:])
            nc.scalar.dma_start(*xkg(0))
            nc.sync.dma_start(*xkg(1))
            nc.scalar.dma_start(*xkg(2))
            nc.sync.dma_start(*xkg(3))
            nc.scalar.dma_start(wq[:, 0], wq_d[:, 0])
            nc.sync.dma_start(wq[:, 1], wq_d[:, 1])
            nc.scalar.dma_start(wq[:, 2], wq_d[:, 2])
            nc.sync.dma_start(wq[:, 3], wq_d[:, 3])
            nc.gpsimd.dma_start(wv[:], wv_d[:])
            nc.gpsimd.dma_start(id_sb[:], id_d[:])
            nc.gpsimd.dma_start(mask_sb[:], mask_d[:])
            nc.sync.dma_start(wo[:, 0:2, :], wo_d[:, 0:2, :])
            nc.scalar.dma_start(wo[:, 2:4, :], wo_d[:, 2:4, :])
            nc.scalar.dma_start(xT[:, 1, 0:8, :], xT_d[:, 1, 0:8, :])
            nc.sync.dma_start(xT[:, 1, 8:16, :], xT_d[:, 1, 8:16, :])
            nc.sync.dma_start(xT[:, 2], xT_d[:, 2])
            nc.scalar.dma_start(xT[:, 3], xT_d[:, 3])

            ones_sb = const.tile([128, 128], bf16, tag="ones")
            nc.vector.memset(ones_sb[:], 1.0)

            # rope tables via angle addition (no big table loads, no Sin):
            # cos(A+B) = cosA*cosB - sinA*sinB with A = invf*512c (host
            # scalars in rota) and B = invf*u (host [128,512] tables).
            Mul = mybir.AluOpType.mult
            for c in range(NCQ):
                tcs = slice(c * 512, (c + 1) * 512)
                t1 = rope_pool.tile([128, 512], f32, tag="qf", name=f"tc{c}a")
                t2 = rope_pool.tile([128, 512], f32, tag="qp", name=f"tc{c}b")
                nc.vector.tensor_scalar(t1[:], cosb_sb[:],
                                        rota_sb[:, 0, c:c+1], None, op0=Mul)
                nc.vector.tensor_scalar(t2[:], sinb_sb[:],
                                        rota_sb[:, 1, c:c+1], None, op0=Mul)
                nc.vector.tensor_sub(cos_sb[:, tcs], t1[:], t2[:])
                t3 = rope_pool.tile([128, 512], f32, tag="qf", name=f"tc{c}c")
                t4 = rope_pool.tile([128, 512], f32, tag="qp", name=f"tc{c}d")
                nc.vector.tensor_scalar(t3[:], cosb_sb[:],
                                        rota_sb[:, 3, c:c+1], None, op0=Mul)
                nc.vector.tensor_scalar(t4[:], sinb_sb[:],
                                        rota_sb[:, 2, c:c+1], None, op0=Mul)
                nc.vector.tensor_add(ssw_sb[:, tcs], t3[:], t4[:])

            # persistent activations
            qT = qkv.tile([128, HEADS_PER_CORE, t_len], bf16, tag="qT")
            kT = qkv.tile([128, t_len], bf16, tag="kT")
            v_sb = qkv.tile([128, NT, HD], bf16, tag="v")

            def rope_to(dst_ap, psum_tile, c):
                """RoPE a [128, 512] psum tile (rows = [even|odd] dims of one
                head, cols = T positions of chunk c) into bf16 dst_ap."""
                cs = slice(c * 512, (c + 1) * 512)
                qf = rope_pool.tile([128, 512], f32, tag="qf")
                qp = rope_pool.tile([128, 512], f32, tag="qp")
                qs = rope_pool.tile([128, 512], f32, tag="qs")
                nc.vector.tensor_mul(qf[:], psum_tile[:], cos_sb[:, cs])
                nc.vector.tensor_mul(qp[:], psum_tile[:], ssw_sb[:, cs])
                nc.gpsimd.dma_start(qs[0:64, :], qp[64:128, :])
                nc.gpsimd.dma_start(qs[64:128, :], qp[0:64, :])
                nc.vector.tensor_add(dst_ap, qf[:], qs[:])

            def kv_proj(c):
                """k^T and v projections for chunk c.  For chunk 0 the k/v
                matmuls interleave by k-group (the runtime PE queue is
                strictly in-order, so work must be emitted in data-arrival
                order) with dummy pads covering the DMA pacing."""
                cs = slice(c * 512, (c + 1) * 512)
                kp = psum.tile([128, 512], f32, tag="proj", bufs=2)
                vp = psum.tile([128, 512], f32, tag="proj", bufs=2)
                for k in range(NK):
                    nc.tensor.matmul(kp[:], wk[:, k, :], xT[:, c, k, :],
                                     start=(k == 0), stop=(k == NK - 1))
                for k in range(NK):
                    nc.tensor.matmul(vp[:], wv[:, k, :], xT[:, c, k, :],
                                     start=(k == 0), stop=(k == NK - 1))
                rope_to(kT[:, cs], kp, c)
                vT_sb = rope_pool.tile([128, 512], bf16, tag="vT")
                nc.any.tensor_copy(out=vT_sb[:], in_=vp[:])
                for tt in range(4):
                    trp_full = psum.tile([128, 512], f32, tag="proj", bufs=2,
                                         name="trp")
                    trp = trp_full.bitcast(bf16)[:, :128]
                    nc.tensor.transpose(trp, vT_sb[:, ts(tt, 128)], id_sb[:])
                    nc.any.tensor_copy(out=v_sb[:, 4 * c + tt, :], in_=trp)

            def q_proj(c, h):
                cs = slice(c * 512, (c + 1) * 512)
                qp = psum.tile([128, 512], f32, tag="proj", bufs=2)
                for k in range(NK):
                    nc.tensor.matmul(qp[:], wq[:, h, k, :], xT[:, c, k, :],
                                     start=(k == 0), stop=(k == NK - 1))
                rope_to(qT[:, h, cs], qp, c)

            def attn_head(c, h, attn_t):
                """Attention for q chunk c, head h -> attn_t[:, h, :]."""
                nj = 4 * c + 4
                out_ps = psum.tile([128, 512], f32, tag="out", bufs=1)
                sums_ps = psum.tile([128, 512], f32, tag="sums", bufs=1)
                for j in range(nj):
                    # columns < o*128 of this [tk-tile, q-chunk] block
                    # are fully masked (tk > tq): skip them everywhere
                    o = j - 4 * c
                    lo = max(o, 0) * 128
                    qs0 = c * 512 + lo
                    s_ps = psum.tile([128, 512], f32, tag="s", bufs=4)
                    nc.tensor.matmul(s_ps[:, lo:], kT[:, ts(j, 128)],
                                     qT[:, h, qs0:(c + 1) * 512],
                                     start=True, stop=True)
                    if o >= 0:
                        nc.vector.tensor_add(s_ps[:, lo:lo + 128],
                                             s_ps[:, lo:lo + 128],
                                             mask_sb[:])
                    p = p_pool.tile([128, 512], bf16, tag="p")
                    nc.scalar.activation(p[:, lo:], s_ps[:, lo:], Exp,
                                         bias=0.0, scale=SCALE)
                    nc.tensor.matmul(out_ps[:, lo:], v_sb[:, j, :],
                                     p[:, lo:],
                                     start=(j == 0), stop=(j == nj - 1))
                    nc.tensor.matmul(sums_ps[:, lo:], ones_sb[:],
                                     p[:, lo:],
                                     start=(j == 0), stop=(j == nj - 1))
                rc = recip_pool.tile([128, 512], f32, tag="rc")
                nc.vector.reciprocal_approx_fast(out=rc[:], in_=sums_ps[:])
                nc.vector.tensor_mul(attn_t[:, h, :], out_ps[:], rc[:])

            yqs = [nc.sync, nc.scalar, nc.gpsimd]

            def wo_tile(c, tq, attn_t, split_dma=False):
                """Output projection for row tile tq of q chunk c."""
                row0 = (4 * c + tq) * 128
                # keep the final chunk's DMAs off the slow gpsimd ring so
                # the end-of-kernel queue drain doesn't stretch the tail
                nq = 2 if c == NCQ - 1 else 3
                ysb = y_pool.tile([128, D], bf16, tag="y")
                for half in range(2):
                    # two psum tiles per stationary load: consecutive
                    # matmuls share lhsT so LDWEIGHTS fully hides
                    yp0 = psum.tile([128, 512], f32, tag="s", bufs=4,
                                    name="yp0")
                    yp1 = psum.tile([128, 512], f32, tag="s", bufs=4,
                                    name="yp1")
                    for h in range(HEADS_PER_CORE):
                        for sub, yp in ((0, yp0), (1, yp1)):
                            nn = 2 * half + sub
                            nc.tensor.matmul(yp[:],
                                             attn_t[:, h, ts(tq, 128)],
                                             wo[:, h, ts(nn, 512)],
                                             start=(h == 0), stop=(h == 3))
                    for sub, yp in ((0, yp0), (1, yp1)):
                        nn = 2 * half + sub
                        if split_dma:
                            # tail: copies on alternating engines, ship each
                            # 512-col piece as soon as its copy lands
                            if nn % 2 == 0:
                                nc.vector.tensor_copy(
                                    out=ysb[:, ts(nn, 512)], in_=yp[:])
                            else:
                                nc.scalar.copy(ysb[:, ts(nn, 512)], yp[:])
                            yqs[nn % nq].dma_start(
                                y_d[row0:row0 + 128, ts(nn, 512)],
                                ysb[:, ts(nn, 512)])
                        else:
                            nc.vector.tensor_copy(out=ysb[:, ts(nn, 512)],
                                                  in_=yp[:])
                if not split_dma:
                    yqs[(4 * c + tq) % nq].dma_start(
                        y_d[row0:row0 + 128, :], ysb[:])

            # Emission = scheduler priority order: within a chunk, each
            # head's q-projection immediately precedes its attention so the
            # first exp fires ~35us in; the previous chunk's Wo tiles ride
            # between heads as always-ready PE filler for the ACT-paced
            # attention stretches.
            # Dummy matmuls: always-ready PE filler for the DMA-paced start.
            # They pad the arrival gaps so the HAM clock-gate warms early and
            # stays warm; results are never used (one dummy reader each).
            dmy_in = const.tile([128, 512], bf16, tag="dmy")
            nc.vector.memset(dmy_in[:], 0.0)
            dmy_rd = rope_pool.tile([128, 512], f32, tag="dmyrd")

            def dummy_mms(n, tag):
                dp = psum.tile([128, 512], f32, tag="sums", bufs=1,
                               name=f"dmy{tag}")
                for i in range(n):
                    nc.tensor.matmul(dp[:], ones_sb[:], dmy_in[:],
                                     start=(i == 0), stop=(i == n - 1))
                nc.vector.tensor_copy(out=dmy_rd[:], in_=dp[:])

            attn_ts = [None] * NCQ
            pads = {0: 8, 1: 10, 2: 12, 3: 8}
            for c in range(NCQ):
                cs = slice(c * 512, (c + 1) * 512)
                if c == 0:
                    dummy_mms(14, "warm")
                kv_proj(c)
                if c == 0:
                    dummy_mms(8, "pad0")
                attn_ts[c] = attn_pool.tile([128, HEADS_PER_CORE, 512], bf16,
                                            tag="attnT", name=f"attn_t{c}")
                for h in range(HEADS_PER_CORE):
                    q_proj(c, h)
                    if c == 0:
                        dummy_mms(pads[h], f"padq{h}")
                for h in range(HEADS_PER_CORE):
                    attn_head(c, h, attn_ts[c])
                for tq in range(4):
                    wo_tile(c, tq, attn_ts[c],
                            split_dma=(c == NCQ - 1 and tq == 3))

    nc.finalize()
    return nc


def _prep_inputs(x, Wq, Wk, Wv, Wo, t_len=T):
    """Host-side shard + layout prep -> per-core input maps."""
    import ml_dtypes
    bf16 = ml_dtypes.bfloat16

    x = np.asarray(x, np.float32)
    Wq = np.asarray(Wq, np.float32)
    Wk = np.asarray(Wk, np.float32)
    Wv = np.asarray(Wv, np.float32)
    Wo = np.asarray(Wo, np.float32)

    NK = D // 128
    NCQ = t_len // 512

    # RoPE de-interleave permutation within one head: [evens | odds]
    perm = np.concatenate([np.arange(0, HD, 2), np.arange(1, HD, 2)])

    # rope tables (match reference: freqs = t * base**(-2j/HD)), built on
    # device via angle addition from small host tables:
    #   cos_dup[p, 512c+u] = cosA[p,c]*cosB[p,u] - sinA[p,c]*sinB[p,u]
    #   ssig_sw[p, t] = sgn[p]*sin(invf[p]*t)   (sgn = [+1]*64 + [-1]*64)
    inv = 1.0 / (ROPE_BASE ** (np.arange(0, HD, 2, dtype=np.float32) / HD))
    invd = np.concatenate([inv, inv])                      # [128]
    u = np.arange(512, dtype=np.float64)
    fb = invd[:, None].astype(np.float64) * u[None, :]     # [128, 512]
    cosb = np.cos(fb).astype(bf16)
    sinb = np.sin(fb).astype(bf16)
    sgn = np.concatenate([np.ones(64), -np.ones(64)])
    rota = np.empty((128, 4, t_len // 512), np.float32)
    for c in range(t_len // 512):
        a = invd.astype(np.float64) * (512.0 * c)
        rota[:, 0, c] = np.cos(a)
        rota[:, 1, c] = np.sin(a)
        rota[:, 2, c] = sgn * np.cos(a)
        rota[:, 3, c] = sgn * np.sin(a)

    # strict-lower-triangular causal mask template for the diagonal
    # [tk-tile, tq-tile] block (tk > tq within the 128x128 block)
    r = np.arange(128)[:, None]
    col = np.arange(128)[None, :]
    mask_t = np.where(r > col, MASK_VAL, 0.0).astype(bf16)
    id128 = np.eye(128, dtype=np.float32).astype(bf16)

    def dram_kp(w):  # [D, n] -> [128, NK, n] (partition-major k-chunks)
        n = w.shape[1]
        return np.ascontiguousarray(
            w.reshape(NK, 128, n).transpose(1, 0, 2)).astype(bf16)

    in_maps = []
    for b in range(B):
        xTb = x[b, :t_len].T  # [D, T]
        xTb = np.ascontiguousarray(
            xTb.reshape(NK, 128, NCQ, 512).transpose(1, 2, 0, 3)).astype(bf16)
        for g in range(KV):
            wq_g = Wq[:, g * DQ:(g + 1) * DQ].reshape(D, HEADS_PER_CORE, HD)
            wq_g = wq_g[:, :, perm]  # [D, 4, HD]
            # per-head contiguous layout: [128, h, k, HD]
            wq_g = np.ascontiguousarray(
                wq_g.reshape(NK, 128, HEADS_PER_CORE, HD)
                .transpose(1, 2, 0, 3)).astype(bf16)
            wk_g = Wk[:, g * HD:(g + 1) * HD][:, perm]
            wv_g = Wv[:, g * HD:(g + 1) * HD]
            wo_g = Wo[g * DQ:(g + 1) * DQ, :]  # [512, D]
            wo_g = np.ascontiguousarray(
                wo_g.reshape(HEADS_PER_CORE, 128, D).transpose(1, 0, 2)
            ).astype(bf16)
            in_maps.append({
                "xT": xTb, "wq": wq_g, "wk": dram_kp(wk_g),
                "wv": dram_kp(wv_g), "wo": wo_g, "cosb": cosb,
                "sinb": sinb, "rota": rota, "mask": mask_t, "id128": id128,
            })
    return in_maps


def run(inputs, trace=False, t_len=T):
    """Run the sharded kernel; returns (y_full, BassKernelResults)."""
    from concourse.bass_utils import run_bass_kernel_spmd

    key = ("nc", t_len)
    if key not in _CACHE:
        _CACHE[key] = _build_nc(t_len)
    nc = _CACHE[key]

    in_maps = _prep_inputs(inputs["x"], inputs["Wq"], inputs["Wk"],
                           inputs["Wv"], inputs["Wo"], t_len)
    res = run_bass_kernel_spmd(nc, in_maps, list(range(N_CORES)), trace=trace)

    y = np.empty((B, t_len, D), np.float32)
    for b in range(B):
        acc = np.zeros((t_len, D), np.float32)
        for g in range(KV):
            acc += np.asarray(res.results[b * KV + g]["y"], np.float32)
        y[b] = acc
    return y, res


def kernel(**inputs) -> np.ndarray:
    y, _ = run(inputs, trace=False)
    return y
